# revision 1
# baseline (speedup 1.0000x reference)
"""Bidirectional cross-attention kernel for Trainium2 (8 NeuronCores).

Problem: B=2, N=M=2048, DIM=512, 8 heads x 64 dim_head.
  qk = x @ w_qk, v = x @ w_v, cqk = ctx @ w_cqk, cv = ctx @ w_cv  (per-head)
  sim = qk @ cqk^T * scale                       [b,h,n,m]
  out = softmax_j(sim) @ cv   -> merge -> @ w_out + b_out
  ctx_out = softmax_i(sim)^T @ v -> merge -> @ w_cout + b_cout

Sharding: 16 (batch, head) pairs over 8 cores -> core c handles batch c//4
and heads {2*(c%4), 2*(c%4)+1}. Projection weights are sliced per-core; the
final output projections produce per-core partials which the host sums.

Device algorithm per core (exp computed ONCE per head):
  - project x^T,ctx^T -> qk^T,cqk^T (f32r, scale folded into wq on host) and
    v^T,cv^T -> transposed via PE into bf16 "v-natural" layout with an
    appended ones column per head: [v_h0|1|v_h1|1].
  - per head, E-pass over i-blocks: S = qk^T' cqk^T (f32r matmuls, fp32
    psum), exp on ACT -> E_i [128,1024] bf16; ctx accumulation
    acc_ctx[65, j] += [v|1]' E_i (the ones row accumulates colsum(j));
    each E_i tile is DMA-xbar-transposed into an E^T buffer (reusing the
    x/ctx SBUF slots).
  - out-pass over j-blocks: acc_out[65, i] += [cv|1]' E^T_j (ones row
    accumulates rowsum(i)).
  - normalization: reciprocal of the free-major sums row, GPSIMD
    partition-broadcast, one DVE multiply psum->bf16 outT/ctxT per head.
  - finals: out[i,:] = sum_h outT_h' @ wo_h accumulated in one psum tile
    (bf16 matmuls), plain copy to staging, DMA out. Host adds biases and
    sums the per-core partials.
"""

import os
import numpy as np
from contextlib import ExitStack

_PHASES = int(os.environ.get("KERNEL_PHASES", "9"))

import concourse.bass as bass
import concourse.tile as tile
from concourse import bacc, mybir
from concourse.bass_utils import run_bass_kernel_spmd
from concourse.masks import make_identity

F32 = mybir.dt.float32
F32R = mybir.dt.float32r
BF16 = mybir.dt.bfloat16
AF = mybir.ActivationFunctionType
OP = mybir.AluOpType

B = 2
N = 2048          # sequence length (= context length M)
DIM = 512
P = 128
NT = N // P       # 16 seq tiles
NCH = N // 512    # 4 chunks of 512
KT = DIM // P     # 4 k-tiles
SCALE = 0.125     # 1/sqrt(64)


def build_bass(repeat=1):
    nc = bacc.Bacc("TRN2", debug=False)

    xT = nc.dram_tensor("xT", [DIM, N], F32R, kind="ExternalInput").ap()
    cT = nc.dram_tensor("cT", [DIM, N], F32R, kind="ExternalInput").ap()
    wq = nc.dram_tensor("wq", [DIM, P], F32R, kind="ExternalInput").ap()
    wcq = nc.dram_tensor("wcq", [DIM, P], F32R, kind="ExternalInput").ap()
    wv = nc.dram_tensor("wv", [DIM, P], F32R, kind="ExternalInput").ap()
    wcv = nc.dram_tensor("wcv", [DIM, P], F32R, kind="ExternalInput").ap()
    wo = nc.dram_tensor("wo", [P, DIM], F32, kind="ExternalInput").ap()
    wco = nc.dram_tensor("wco", [P, DIM], F32, kind="ExternalInput").ap()
    outp = nc.dram_tensor("outp", [N, DIM], F32, kind="ExternalOutput").ap()
    ctxp = nc.dram_tensor("ctxp", [N, DIM], F32, kind="ExternalOutput").ap()

    with tile.TileContext(nc) as tc:
        for _rep in range(repeat):
            with ExitStack() as ctx:
                big = ctx.enter_context(tc.tile_pool(name="big", bufs=1))
                small = ctx.enter_context(tc.tile_pool(name="small", bufs=1))
                epool = ctx.enter_context(tc.tile_pool(name="e", bufs=6))
                stage = ctx.enter_context(tc.tile_pool(name="stage", bufs=3))

                ident = small.tile([P, P], F32, tag="ident")
                make_identity(nc, ident[:])

                # ---- load inputs ----
                x_sb = big.tile([P, KT * N], F32R, tag="x")
                c_sb = big.tile([P, KT * N], F32R, tag="c")
                nc.gpsimd.dma_start(
                    x_sb[:].rearrange("p (k n) -> p k n", k=KT),
                    xT.rearrange("(k p) n -> p k n", p=P),
                )
                nc.gpsimd.dma_start(
                    c_sb[:].rearrange("p (k n) -> p k n", k=KT),
                    cT.rearrange("(k p) n -> p k n", p=P),
                )
                w_sbs = {}
                for name, dram in (("wq", wq), ("wcq", wcq), ("wv", wv), ("wcv", wcv)):
                    w_sb = big.tile([P, KT * P], F32R, tag=name)
                    nc.sync.dma_start(
                        w_sb[:].rearrange("p (k m) -> p k m", k=KT),
                        dram.rearrange("(k p) m -> p k m", p=P),
                    )
                    w_sbs[name] = w_sb
                # final-projection weights: [64, DIM] bf16 per head
                wo_sb = {}
                wco_sb = {}
                for h in range(2):
                    for tag, dram, store in (("wo", wo, wo_sb), ("wco", wco, wco_sb)):
                        wf = small.tile([64, DIM], F32, tag=f"{tag}f{h}", name=f"{tag}f{h}")
                        nc.sync.dma_start(wf[:], dram[64 * h : 64 * h + 64, :])
                        wb = small.tile([64, DIM], BF16, tag=f"{tag}b{h}", name=f"{tag}b{h}")
                        nc.vector.tensor_copy(wb[:], wf[:])
                        store[h] = wb

                # ---- projections: qkT/cqkT (f32r), vT/cvT (f32) [128, 2048] ----
                qkT = big.tile([P, N], F32R, tag="qkT")
                cqkT = big.tile([P, N], F32R, tag="cqkT")
                vT = big.tile([P, N], BF16, tag="vT")
                cvT = big.tile([P, N], BF16, tag="cvT")
                vsc = big.tile([P, N], BF16, tag="vsc")
                cvsc = big.tile([P, N], BF16, tag="cvsc")
                # v-natural bf16 with ones columns: per i-block [v_h0|1|v_h1|1]
                vna = big.tile([P, NT * 130], BF16, tag="vna")
                cvna = big.tile([P, NT * 130], BF16, tag="cvna")
                nc.vector.memset(vna[:], 1.0)
                nc.vector.memset(cvna[:], 1.0)

                with tc.tile_pool(name="pp", bufs=2, space="PSUM") as pp:
                    projs = [
                        (w_sbs["wq"], x_sb, qkT, nc.scalar.copy),
                        (w_sbs["wcq"], c_sb, cqkT, nc.vector.tensor_copy),
                        (w_sbs["wv"], x_sb, vT, nc.scalar.copy),
                        (w_sbs["wcv"], c_sb, cvT, nc.vector.tensor_copy),
                    ]
                    for w_sb, src, dst, cp in projs:
                        for ch in range(NCH):
                            ps = pp.tile([P, 512], F32, tag="pj")
                            for kt in range(KT):
                                nc.tensor.matmul(
                                    ps[:],
                                    w_sb[:, kt * P : (kt + 1) * P],
                                    src[:, kt * N + ch * 512 : kt * N + (ch + 1) * 512],
                                    start=(kt == 0),
                                    stop=(kt == KT - 1),
                                )
                            cp(dst[:, ch * 512 : (ch + 1) * 512], ps[:])

                    # v-naturals: xbar-transpose into aligned scratch, then one
                    # strided DVE copy into the 130-column ones layout
                    for vsrc, sc, dst in ((vT, vsc, vna), (cvT, cvsc, cvna)):
                        nc.scalar.dma_start(
                            sc[:].rearrange("p (t s) -> p t s", s=P),
                            vsrc[:],
                            transpose=True,
                        )
                        nc.vector.tensor_copy(
                            dst[:].rearrange("p (g s) -> p g s", s=130 // 2)[:, :, 0:64],
                            sc[:].rearrange("p (g s) -> p g s", s=64),
                        )

                if _PHASES < 2:
                    continue
                # ---- per-head attention: E-pass + out-pass ----
                outT = {}
                ctxT = {}
                for h in range(2):
                    hs = 64 * h
                    with tc.tile_pool(name=f"pacc{h}", bufs=1, space="PSUM") as pacc:
                        acc_ctx = pacc.tile([65, N], F32, tag="acc_ctx", name=f"acc_ctx{h}")
                        # E^T mega buffers reuse the x/c SBUF slots (bf16,
                        # same byte size). Half the j range in each.
                        et = {
                            0: big.tile([P, 8 * N], BF16, tag="x", name=f"et0_{h}"),
                            1: big.tile([P, 8 * N], BF16, tag="c", name=f"et1_{h}"),
                        }
                        with tc.tile_pool(name=f"ps{h}", bufs=2, space="PSUM") as psp:
                            # software-pipelined emission: S/exp of step i are
                            # emitted before ctx-MMs/transpose of step i-1 so
                            # the PE prioritizes feeding the next exp.
                            def _consume(e, t, jh):
                                for q in range(2):
                                    ch = jh * 2 + q
                                    nc.tensor.matmul(
                                        acc_ctx[:, ch * 512 : (ch + 1) * 512],
                                        vna[:, t * 130 + 65 * h : t * 130 + 65 * h + 65],
                                        e[:, q * 512 : (q + 1) * 512],
                                        start=(t == 0),
                                        stop=(t == NT - 1),
                                    )
                                # transpose E_i(j-half) into E^T blocks
                                # (scalar HWDGE ring, parallel to sync ring)
                                nc.scalar.dma_start(
                                    et[jh][:]
                                    .rearrange("p (b n) -> p b n", b=8)
                                    [:, :, t * P : (t + 1) * P],
                                    e[:],
                                    transpose=True,
                                )

                            pend = []
                            for t in range(NT):
                                for jh in range(2):
                                    sp = psp.tile([P, 1024], F32, tag="s")
                                    for q in range(2):
                                        ch = jh * 2 + q
                                        nc.tensor.matmul(
                                            sp[:, q * 512 : (q + 1) * 512],
                                            qkT[hs : hs + 64, t * P : (t + 1) * P],
                                            cqkT[hs : hs + 64, ch * 512 : (ch + 1) * 512],
                                            start=True,
                                            stop=True,
                                        )
                                    e = epool.tile([P, 1024], BF16, tag="e")
                                    nc.scalar.activation(e[:], sp[:], AF.Exp)
                                    pend.append((e, t, jh))
                                    if len(pend) > 1:
                                        _consume(*pend.pop(0))
                            for args in pend:
                                _consume(*args)

                        if _PHASES < 3:
                            continue

                        def _normalize(acc, store, nm):
                            rr = small.tile([1, N], F32, tag="rr", name=f"rr{nm}{h}")
                            nc.vector.reciprocal(rr[:], acc[64:65, :])
                            rb = small.tile([64, N], F32, tag="rb", name=f"rb{nm}{h}")
                            nc.gpsimd.partition_broadcast(rb[:], rr[:])
                            dst = small.tile([64, N], BF16, tag=f"dT{nm}{h}", name=f"dT{nm}{h}")
                            nc.vector.tensor_tensor(
                                dst[:], acc[0:64, :], rb[:], op=OP.mult
                            )
                            store[h] = dst

                        # out-pass: acc_out[65, i] += [cv|1]' E^T_j
                        with tc.tile_pool(name=f"po{h}", bufs=1, space="PSUM") as po:
                            acc_out = po.tile([65, N], F32, tag="acc_out", name=f"acc_out{h}")
                            _normalize(acc_ctx, ctxT, "c")
                            for jb in range(NT):
                                mega = et[0] if jb < 8 else et[1]
                                base = (jb % 8) * N
                                for q in range(NCH):
                                    nc.tensor.matmul(
                                        acc_out[:, q * 512 : (q + 1) * 512],
                                        cvna[:, jb * 130 + 65 * h : jb * 130 + 65 * h + 65],
                                        mega[:, base + q * 512 : base + (q + 1) * 512],
                                        start=(jb == 0),
                                        stop=(jb == NT - 1),
                                    )

                            _normalize(acc_out, outT, "o")

                if _PHASES < 4:
                    continue
                # ---- final projections (heads merged in psum) ----
                with tc.tile_pool(name="pf", bufs=4, space="PSUM") as pf:
                    finals = [
                        (outT, wo_sb, outp),
                        (ctxT, wco_sb, ctxp),
                    ]
                    for srcT, W, dram in finals:
                        dram_r = dram.rearrange("(g q p) d -> g p q d", p=P, q=4)
                        for g in range(4):
                            st = stage.tile([P, 4 * DIM], F32, tag="st")
                            for q in range(4):
                                it = g * 4 + q
                                f = pf.tile([P, DIM], F32, tag="f")
                                nc.tensor.matmul(
                                    f[:],
                                    srcT[0][:, it * P : (it + 1) * P],
                                    W[0][:],
                                    start=True,
                                    stop=False,
                                )
                                nc.tensor.matmul(
                                    f[:],
                                    srcT[1][:, it * P : (it + 1) * P],
                                    W[1][:],
                                    start=False,
                                    stop=True,
                                )
                                cp = nc.scalar.copy if q % 2 == 0 else nc.vector.tensor_copy
                                cp(st[:, q * DIM : (q + 1) * DIM], f[:])
                            eng = nc.sync if g % 2 == 0 else nc.scalar
                            eng.dma_start(
                                dram_r[g],
                                st[:].rearrange("p (q d) -> p q d", q=4),
                            )

    nc.compile()
    return nc


_NC = None


def _get_nc():
    global _NC
    if _NC is None:
        _NC = build_bass()
    return _NC


def make_in_maps(x, context, w_qk, w_cqk, w_v, w_cv, w_out, w_cout):
    x = np.asarray(x, np.float32)
    context = np.asarray(context, np.float32)
    xTs = [np.ascontiguousarray(x[b].T) for b in range(B)]
    cTs = [np.ascontiguousarray(context[b].T) for b in range(B)]
    in_maps = []
    for c in range(8):
        b = c // 4
        hs = (c % 4) * P
        in_maps.append(
            {
                "xT": xTs[b],
                "cT": cTs[b],
                "wq": np.ascontiguousarray(
                    np.asarray(w_qk, np.float32)[:, hs : hs + P] * SCALE
                ),
                "wcq": np.ascontiguousarray(np.asarray(w_cqk, np.float32)[:, hs : hs + P]),
                "wv": np.ascontiguousarray(np.asarray(w_v, np.float32)[:, hs : hs + P]),
                "wcv": np.ascontiguousarray(np.asarray(w_cv, np.float32)[:, hs : hs + P]),
                "wo": np.ascontiguousarray(np.asarray(w_out, np.float32)[hs : hs + P, :]),
                "wco": np.ascontiguousarray(np.asarray(w_cout, np.float32)[hs : hs + P, :]),
            }
        )
    return in_maps


def gather(results, b_out, b_cout):
    out = np.zeros((B, N, DIM), np.float32)
    ctx_out = np.zeros((B, N, DIM), np.float32)
    for c in range(8):
        out[c // 4] += results[c]["outp"]
        ctx_out[c // 4] += results[c]["ctxp"]
    out += np.asarray(b_out, np.float32)
    ctx_out += np.asarray(b_cout, np.float32)
    return out, ctx_out


def kernel(x, context, w_qk, w_cqk, w_v, w_cv, w_out, b_out, w_cout, b_cout):
    nc = _get_nc()
    in_maps = make_in_maps(x, context, w_qk, w_cqk, w_v, w_cv, w_out, w_cout)
    res = run_bass_kernel_spmd(nc, in_maps, core_ids=list(range(8)))
    return gather(res.results, b_out, b_cout)

